# revision 19
# baseline (speedup 1.0000x reference)
"""DecoderRNN (embedding + LSTM recurrence + vocab projection) on 8 Trainium2
NeuronCores.

Problem: B=128, T=20, E=H=512, V=10000
    x = embed_W[captions]            # [B, T, E]
    LSTM over T steps from h0=features, c0=0 (PyTorch LSTMCell gate order)
    out[t] = h_t @ lin_W.T + lin_b   # logits [B, T, V]

Sharding (8 cores = 4 batch-groups x 2 vocab-halves, no collectives):
    core c -> batch rows  [32*(c//2), 32*(c//2)+32)  (recurrence duplicated 2x)
              vocab cols  [5000*(c%2), 5000*(c%2)+5000)
Host scatters inputs (pre-transposed/packed, weights cast to bf16) and gathers
the per-core output blocks. Matmuls run in bf16 with fp32 PSUM accumulation;
the cell state and all nonlinearities stay fp32. Logits are stored/shipped as
fp16 (adds ~5e-4 rel err; tolerance is 2e-2) and upcast to fp32 on the host.

Device scheme (per core, per step):
  - All activations are kept as the matmul *stationary* operand (lhsT):
    hT[p, q, b] = h[b, q*128+p] (bf16). The LSTM gate matmuls and the vocab
    projection both column-tile the 128x128 PE array 4 ways (M=32 per
    col-group), so the array stays fully utilized despite the small per-core
    batch.
  - Gates land in PSUM packed as [128=(q,b), 512=(slot,h_lo)] with slot order
    (i, f, o, g), h = q*128 + h_lo. This gives full-partition elementwise ops:
    one sigmoid over [128,384], one tanh over [128,128], DVE cell update on
    [128,128] tiles.
  - h is re-transposed each step on the DVE as a single 32x32 block-stream
    transpose of the bf16 h tile. The missing inter-block permutation is
    absorbed into a host-side permutation of the H-contraction rows of
    W_hh / lin_W / features: with hidden index h = 128*G + 32*F + i, the
    block transpose puts h on partition 32*G + i of contraction chunk F, so
    the host packs chunk F's weight rows in (G, i) order.
  - The x-contribution of step t+1's gates is issued to the PE right after
    step t's h-contribution, so the PE computes it (and the projection of
    step t) while ACT/DVE run step t's nonlinearities.
  - Projection loops k (contraction chunk) outermost so the 3 output blocks
    share each stationary load; results are staged in SBUF as fp16 and
    written back with one large DMA per S_OUT steps instead of 12 small
    DMAs per step.
"""

import numpy as np
from contextlib import ExitStack

import ml_dtypes
import concourse.bass as bass
import concourse.mybir as mybir
import concourse.tile as tile
from concourse import bacc
from concourse.bass import ds, ts
from concourse.bass_utils import run_bass_kernel_spmd

B, T, E, H, V = 128, 20, 512, 512, 10000
NB, NV = 4, 2                    # batch groups x vocab halves
BL, VL = B // NB, V // NV        # 32 rows, 5000 vocab cols per core
N_CORES = NB * NV
KE, KH = E // 128, H // 128      # 4, 4 contraction chunks
QV = VL // 4                     # 1250 vocab cols per col-group
S_OUT = 10                       # steps staged in SBUF per output DMA
F32 = mybir.dt.float32
F16 = mybir.dt.float16
BF16 = mybir.dt.bfloat16
NP_BF16 = ml_dtypes.bfloat16

# projection blocks per col-group: 1250 = 512 + 512 + 226 (one PSUM bank each)
PROJ_BLOCKS = [(0, 512), (512, 512), (1024, 226)]

SIG = mybir.ActivationFunctionType.Sigmoid
TANH = mybir.ActivationFunctionType.Tanh

_BUILD_CACHE = {}


def _build(gbias_nz: bool, lbias_nz: bool, reps: int = 1):
    """Build + compile the per-core SPMD program (reps>1 only for timing)."""
    nc = bacc.Bacc("TRN2", target_bir_lowering=False, debug=False)

    wT_d = nc.dram_tensor("wT", [128, KE + KH, 4, 512], BF16, kind="ExternalInput").ap()
    linT_d = nc.dram_tensor("linT", [128, KH, VL], BF16, kind="ExternalInput").ap()
    xT_d = nc.dram_tensor("xT", [128, T, KE, BL], BF16, kind="ExternalInput").ap()
    hT0_d = nc.dram_tensor("hT0", [128, KH * BL], BF16, kind="ExternalInput").ap()
    gb_d = (
        nc.dram_tensor("gb", [128, 512], F32, kind="ExternalInput").ap()
        if gbias_nz else None
    )
    lb_d = (
        nc.dram_tensor("lb", [128, QV], F32, kind="ExternalInput").ap()
        if lbias_nz else None
    )
    # output keeps the on-chip row packing: row (q*BL + b) covers vocab
    # [q*QV, (q+1)*QV) for batch row b; the host unscrambles. Each per-DMA
    # slice [:, t0:t1, :] is then contiguous per row (S_OUT*QV*2 bytes).
    out_qb = nc.dram_tensor("out", [128, T, QV], F16, kind="ExternalOutput").ap()

    with tile.TileContext(nc) as tc, ExitStack() as ctx:
        consts = ctx.enter_context(tc.tile_pool(name="consts", bufs=1))
        state = ctx.enter_context(tc.tile_pool(name="state", bufs=3))
        work = ctx.enter_context(tc.tile_pool(name="work", bufs=3))
        outp = ctx.enter_context(tc.tile_pool(name="outp", bufs=2 if S_OUT < T else 1))
        psg = ctx.enter_context(tc.tile_pool(name="psg", bufs=2, space="PSUM"))
        psp = ctx.enter_context(tc.tile_pool(name="psp", bufs=6, space="PSUM"))

        # startup DMAs ride two HWDGE rings so nothing queues behind the bulk
        # weight load: sync ring carries x then w (x-chunks first -- step 0's
        # first matmuls need x + w[0]); scalar ring carries hT0 (in-loop) and
        # lin (not needed until the first projection, one DMA per k chunk)
        w_sb = consts.tile([128, KE + KH, 4, 512], BF16, tag="w")
        x_sb = consts.tile([128, T, KE, BL], BF16, tag="x")
        nc.sync.dma_start(x_sb[:, 0:6, :, :], xT_d[:, 0:6, :, :])
        for k in range(KE + KH):
            nc.sync.dma_start(w_sb[:, k, :, :], wT_d[:, k, :, :])
        nc.sync.dma_start(x_sb[:, 6:T, :, :], xT_d[:, 6:T, :, :])
        lin_sb = consts.tile([128, KH, 4, QV], BF16, tag="lin")
        for k in range(KH):
            nc.scalar.dma_start(
                lin_sb[:, k, :, :],
                linT_d[:, k, :].rearrange("p (q j) -> p q j", q=4),
            )

        gb_sb = None
        if gb_d is not None:
            gb_sb = consts.tile([128, 512], F32, tag="gb")
            nc.sync.dma_start(gb_sb[:], gb_d[:])
        lb_sb = None
        if lb_d is not None:
            lb_sb = consts.tile([128, QV], F32, tag="lb")
            nc.sync.dma_start(lb_sb[:], lb_d[:])

        def gates_mm(psum, lhsT_of_k, ks, start):
            """Accumulate sum_k lhsT_k.T @ W_k into the 4 col-groups of psum.

            The has_written clear of `start=True` is per-written-element, so
            each col-group's first matmul must carry start=True (HW-verified:
            a single bank-wide start leaves stale PSUM in the other groups).
            """
            for j, k in enumerate(ks):
                lhsT = lhsT_of_k(k)
                for q in range(4):
                    nc.tensor.matmul(
                        psum[ts(q, 32), :],
                        lhsT,
                        w_sb[:, k, q, :],
                        start=start and j == 0,
                        stop=(not start) and j == len(ks) - 1,
                        skip_group_check=True,
                        tile_position=(0, 32 * q),
                    )

        def emit_proj(t, hT_t, o_stage):
            """Projection for step t: col-group q covers vocab [q*QV, (q+1)*QV).

            k outermost so all 3 blocks reuse the same stationary hT chunk;
            results go to the fp16 staging tile at slot t % S_OUT.
            """
            psums = [
                psp.tile([128, 512], F32, tag="pp", name=f"pp{bi}")
                for bi in range(3)
            ]
            for k in range(KH):
                for bi, (off, n) in enumerate(PROJ_BLOCKS):
                    for q in range(4):
                        nc.tensor.matmul(
                            psums[bi][ts(q, 32), 0:n],
                            hT_t[:, ds(32 * k, 32)],
                            lin_sb[:, k, q, ds(off, n)],
                            start=(k == 0),
                            stop=(k == KH - 1),
                            skip_group_check=True,
                            tile_position=(0, 32 * q),
                        )
            # GPSIMD cannot read PSUM, so evacuate on DVE/ACT; emit_proj is
            # issued after each step's nonlinearity chain so these copies
            # queue behind the critical ops in the strict engine FIFOs
            for bi, (off, n) in enumerate(PROJ_BLOCKS):
                dst = o_stage[:, t % S_OUT, ds(off, n)]
                if lb_sb is not None:
                    nc.vector.tensor_add(dst, psums[bi][:, 0:n], lb_sb[:, ds(off, n)])
                elif bi == 0:
                    nc.vector.tensor_copy(dst, psums[bi][:, 0:n])
                else:
                    nc.scalar.copy(dst, psums[bi][:, 0:n])

        for _rep in range(reps):
            # --- initial state ---
            hT = state.tile([128, KH * BL], BF16, tag="hT")
            nc.scalar.dma_start(hT[:], hT0_d[:])
            c_cur = state.tile([128, 128], F32, tag="c")
            nc.vector.memset(c_cur[:], 0.0)

            # x-part of step0 gates
            psum_g = psg.tile([128, 512], F32, tag="pg")
            gates_mm(psum_g, lambda k: x_sb[:, 0, k, :], range(KE), start=True)

            o_stage = None
            for t in range(T):
                # ---- h-part of gates: psum_g += sum_k hT_k.T @ W_hh_k
                gates_mm(
                    psum_g, lambda k: hT[:, ds(32 * (k - KE), 32)],
                    range(KE, KE + KH), start=False,
                )
                if gb_sb is not None:
                    nc.vector.tensor_add(psum_g[:], psum_g[:], gb_sb[:])

                # ---- x-part of next step's gates (keeps PE busy during the
                #      elementwise chain below)
                if t + 1 < T:
                    psum_g_next = psg.tile([128, 512], F32, tag="pg")
                    gates_mm(
                        psum_g_next, lambda k: x_sb[:, t + 1, k, :],
                        range(KE), start=True,
                    )

                # ---- nonlinearities; packed free dim = (slot, h_lo),
                #      slots = (i, f, o, g). sigmoid(i,f) + tanh(g) gate the
                #      cell update; sigmoid(o) is only needed after tanh(c)
                s_ifo = work.tile([128, 384], F32, tag="s_ifo")
                nc.scalar.activation(s_ifo[:, 0:256], psum_g[:, 0:256], SIG)
                g_t = work.tile([128, 128], F32, tag="g_t")
                nc.scalar.activation(g_t[:], psum_g[:, 384:512], TANH)
                nc.scalar.activation(s_ifo[:, 256:384], psum_g[:, 256:384], SIG)

                # ---- cell update (DVE, fp32, [128,128] tiles)
                t1 = work.tile([128, 128], F32, tag="t1")
                nc.vector.tensor_mul(t1[:], s_ifo[:, 128:256], c_cur[:])
                t2 = work.tile([128, 128], F32, tag="t2")
                nc.vector.tensor_mul(t2[:], s_ifo[:, 0:128], g_t[:])
                c_new = state.tile([128, 128], F32, tag="c")
                nc.vector.tensor_add(c_new[:], t1[:], t2[:])
                tc_t = work.tile([128, 128], F32, tag="tc")
                nc.scalar.activation(tc_t[:], c_new[:], TANH)
                h_new = work.tile([128, 128], BF16, tag="h")
                nc.vector.tensor_mul(h_new[:], s_ifo[:, 256:384], tc_t[:])

                # ---- transpose h back to lhsT form on the DVE: 32x32 block
                #      stream transpose; the inter-block permutation is folded
                #      into the host-side weight packing (see module docstring)
                hT_new = state.tile([128, KH * BL], BF16, tag="hT")
                for f in range(KH):
                    nc.vector.transpose(
                        hT_new[:, ds(32 * f, 32)], h_new[:, ds(32 * f, 32)]
                    )

                # ---- projection of the PREVIOUS step (PE runs it while
                #      ACT/DVE execute the chain above; issued last so its
                #      PSUM-evacuation copies follow the chain in the FIFOs)
                if t > 0:
                    if (t - 1) % S_OUT == 0:
                        o_stage = outp.tile([128, S_OUT, QV], F16, tag="o")
                    emit_proj(t - 1, hT, o_stage)
                    if t % S_OUT == 0:
                        # slots 0..S_OUT-1 (steps t-S_OUT..t-1) are complete:
                        # one DMA covers S_OUT steps
                        nc.sync.dma_start(out_qb[:, t - S_OUT : t, :], o_stage[:])
                    elif t == T - 1:
                        # drain all but the final step early so the end-of-
                        # kernel DMA only covers step T-1
                        ns = (t - 1) % S_OUT + 1
                        nc.sync.dma_start(
                            out_qb[:, T - S_OUT : T - S_OUT + ns, :],
                            o_stage[:, 0:ns, :],
                        )

                hT = hT_new
                c_cur = c_new
                if t + 1 < T:
                    psum_g = psum_g_next

            # projection of the final step + tail DMA (one step's worth)
            emit_proj(T - 1, hT, o_stage)
            nc.sync.dma_start(
                out_qb[:, T - 1 : T, :],
                o_stage[:, (T - 1) % S_OUT : (T - 1) % S_OUT + 1, :],
            )

    nc.compile()
    return nc


# gate slot order in the packed layout: (i, f, o, g); rows of W per original
# PyTorch order (i, f, g, o) -> slot permutation:
_SLOT_PERM = [0, 1, 3, 2]

# H-contraction permutation induced by the on-chip 32x32 block transpose of h:
# hidden index 128*G + 32*F + i lands on partition 32*G + i of chunk F, so
# position (chunk k, partition p) reads hidden row 128*(p//32) + 32*k + p%32.
_HIDX = np.arange(512)
_HPERM = 128 * ((_HIDX % 128) // 32) + 32 * (_HIDX // 128) + (_HIDX % 32)


def _pack_w(Wsrc):
    """[4H, 512] -> [128(p), 4(k), 4(q), 512(slot,h_lo)] bf16 with
    w[p,k,q,s*128+hl] = Wsrc[gate(s)*512 + q*128 + hl, k*128 + p]."""
    a = Wsrc.reshape(4, 4, 128, 4, 128)           # (g, q, hl, k, p)
    a = a[_SLOT_PERM]                             # (s, q, hl, k, p)
    a = a.transpose(4, 3, 1, 0, 2)                # (p, k, q, s, hl)
    return np.ascontiguousarray(a.reshape(128, 4, 4, 512)).astype(NP_BF16)


def _host_prep(features, captions, embed_W, W_ih, W_hh, b_ih, b_hh, lin_W, lin_b):
    """Build the 8 per-core input maps (numpy only)."""
    features = np.asarray(features, np.float32)
    captions = np.asarray(captions)
    embed_W = np.asarray(embed_W, np.float32)
    W_ih = np.asarray(W_ih, np.float32)
    W_hh = np.asarray(W_hh, np.float32)
    b_ih = np.asarray(b_ih, np.float32)
    b_hh = np.asarray(b_hh, np.float32)
    lin_W = np.asarray(lin_W, np.float32)
    lin_b = np.asarray(lin_b, np.float32)

    # W_hh / lin_W / features have their H-contraction axis permuted to match
    # the on-chip block-transposed h layout (see _HPERM)
    wT = np.ascontiguousarray(
        np.concatenate([_pack_w(W_ih), _pack_w(W_hh[:, _HPERM])], axis=1)
    )  # [128, 8, 4, 512]

    linT_halves = []
    for vh in range(NV):
        lw = lin_W[vh * VL : (vh + 1) * VL][:, _HPERM]
        linT_halves.append(
            np.ascontiguousarray(lw.T.reshape(KH, 128, VL).transpose(1, 0, 2))
            .astype(NP_BF16)
        )

    x_all = embed_W[captions]  # [B, T, E] fp32 (host-side gather)

    gb = b_ih + b_hh
    gbias_nz = bool(np.any(gb))
    lbias_nz = bool(np.any(lin_b))

    gb_packed = None
    if gbias_nz:
        # gb_packed[(q,b), s*128+hl] = gb[gate(s)*512 + q*128 + hl]
        a = gb.reshape(4, 4, 128)[_SLOT_PERM]      # (s, q, hl)
        a = a.transpose(1, 0, 2).reshape(4, 512)   # (q, (s,hl))
        gb_packed = np.ascontiguousarray(
            np.repeat(a[:, None, :], BL, axis=1).reshape(128, 512)
        ).astype(np.float32)

    in_maps = []
    for core in range(N_CORES):
        bg, vh = divmod(core, NV)
        xe = x_all[bg * BL : (bg + 1) * BL]        # [BL, T, E]
        xT = np.ascontiguousarray(
            xe.reshape(BL, T, KE, 128).transpose(3, 1, 2, 0)
        ).astype(NP_BF16)                          # [128, T, KE, BL]
        feat = features[bg * BL : (bg + 1) * BL][:, _HPERM]
        hT0 = np.ascontiguousarray(
            feat.reshape(BL, KH, 128).transpose(2, 1, 0).reshape(128, KH * BL)
        ).astype(NP_BF16)
        m = {"wT": wT, "linT": linT_halves[vh], "xT": xT, "hT0": hT0}
        if gbias_nz:
            m["gb"] = gb_packed
        if lbias_nz:
            lb = lin_b[vh * VL : (vh + 1) * VL].reshape(4, QV)  # (q, j)
            m["lb"] = np.ascontiguousarray(
                np.repeat(lb[:, None, :], BL, axis=1).reshape(128, QV)
            ).astype(np.float32)
        in_maps.append(m)
    return in_maps, gbias_nz, lbias_nz


def kernel(features, captions, embed_W, W_ih, W_hh, b_ih, b_hh, lin_W, lin_b):
    in_maps, gbias_nz, lbias_nz = _host_prep(
        features, captions, embed_W, W_ih, W_hh, b_ih, b_hh, lin_W, lin_b
    )
    key = (gbias_nz, lbias_nz)
    if key not in _BUILD_CACHE:
        _BUILD_CACHE[key] = _build(*key)
    nc = _BUILD_CACHE[key]

    res = run_bass_kernel_spmd(nc, in_maps, core_ids=list(range(N_CORES)))

    out = np.empty((B, T, V), np.float32)
    for core in range(N_CORES):
        bg, vh = divmod(core, NV)
        o = res.results[core]["out"]  # [128, T, QV] f16, row = q*BL + b
        o = o.reshape(4, BL, T, QV).transpose(1, 2, 0, 3).reshape(BL, T, VL)
        out[bg * BL : (bg + 1) * BL, :, vh * VL : (vh + 1) * VL] = o.astype(
            np.float32
        )
    return out
